# revision 16
# baseline (speedup 1.0000x reference)
"""Trainium2 Bass kernel for nn_Decoder (multihot-embedding -> GRU -> vocab logits).

Strategy (8 NeuronCores):
  K1: batch-parallel embedding stage. Each core handles B/8 = 4 batch rows:
      indirect-DMA gathers of embedding rows (host-deduped multihot weights),
      weighted-sum via small PE matmuls, tanh, then the (cells, 3H) input-gate
      matmul gi = emb @ W_ih.T + (b_ih + b_hh) in full fp32.
  K2: replicated GRU (all cores run the identical B=32 recurrence) + vocab-
      sharded logits/sigmoid (each core computes its V/8 = 2500 W_out slice).
      The GRU's W_hh @ h matmul runs as a 3-term fp32r hi/lo split (bf16-based
      split of W on the host) giving ~fp32 accuracy at 1 cyc/row. The logits
      matmul runs in bf16 (p_output tolerance is orders of magnitude above its
      error; exact ordering is restored on the host).
  Host: shards/reshapes inputs, dedups multihot indices, assembles p_output,
      computes m_output from `length`, and finalizes s_output by top-k over the
      kernel's p_output with an fp64 rerank of candidate logits (built from the
      kernel's fp32-accurate GRU outputs).
"""
import sys

sys.path.insert(0, "/opt/trn_rl_repo")
import numpy as np
import ml_dtypes

import concourse.bass as bass
import concourse.mybir as mybir
from concourse.tile import TileContext
from concourse.masks import make_identity
from concourse.bass_utils import run_bass_kernel_spmd

import bass_rust

B, S, MV = 32, 64, 30
V, E, H, L = 20000, 256, 512, 128
PAD_IDX = 0
NC = 8
BL = B // NC             # batches per core in K1 (4)
CELLS1 = BL * S          # cells per core in K1 (256)
SLOT = 32                # padded gather slots per cell
CHUNKS1 = CELLS1 * SLOT // 128  # 64 gather chunks of 128 rows
VSL = V // NC            # vocab slice per core in K2 (2500)
CELLS = B * S            # 2048; K2 cell order is s-major: cell = s*B + b
F32 = mybir.dt.float32
F32R = mybir.dt.float32r
BF16 = mybir.dt.bfloat16
I32 = mybir.dt.int32
AF = mybir.ActivationFunctionType
OP = mybir.AluOpType


def _legalize_waits(nc, max_waits=1):
    """This walrus build encodes one sync-wait per 64B instruction and rejects
    more; hoist extra waits onto per-engine NoOps inserted just before."""
    n = 0
    for fn in nc.m.functions:
        for bb in fn.blocks:
            out, changed = [], False
            for inst in bb.instructions:
                si = inst.sync_info
                waits = list(si.on_wait) if si is not None and si.on_wait else []
                if len(waits) > max_waits:
                    for j, w in enumerate(waits[:-max_waits]):
                        nop = bass_rust.InstNoOp(name=f"{inst.name}-lw{j}", engine=inst.engine)
                        nop.sync_info = mybir.SyncInfo(on_wait=[w], on_update=[])
                        out.append(nop)
                    inst.sync_info = mybir.SyncInfo(
                        on_wait=waits[-max_waits:], on_update=list(si.on_update or [])
                    )
                    n += 1
                    changed = True
                out.append(inst)
            if changed:
                bb.instructions = out
    return n


# ------------------------------------------------------------------ K1
def build_k1():
    nc = bass.Bass("TRN2", target_bir_lowering=False)
    emb_w = nc.declare_dram_parameter("emb_w", [V, E], F32, isOutput=False)
    idx = nc.declare_dram_parameter("idx", [128, CHUNKS1], I32, isOutput=False)
    wmask = nc.declare_dram_parameter("wmask", [128, CHUNKS1 * 4], F32, isOutput=False)
    wiht = nc.declare_dram_parameter("wiht", [E + 1, 3 * H], F32, isOutput=False)
    gi_out = nc.declare_dram_parameter("gi_out", [CELLS1, 3 * H], F32, isOutput=True)

    GRP = 8  # gather chunks per G tile
    with TileContext(nc) as tc:
        with tc.tile_pool(name="sb", bufs=1) as sb, \
             tc.tile_pool(name="gp", bufs=4) as gp, \
             tc.tile_pool(name="pse", bufs=1, space="PSUM") as pse, \
             tc.tile_pool(name="psg", bufs=1, space="PSUM") as psg, \
             tc.tile_pool(name="pst", bufs=2, space="PSUM") as pst:
            idx_t = sb.tile([128, CHUNKS1], I32)
            nc.sync.dma_start(out=idx_t[:], in_=idx[:])
            wm_t = sb.tile([128, CHUNKS1 * 4], F32)
            nc.sync.dma_start(out=wm_t[:], in_=wmask[:])
            wih_a = sb.tile([128, 3 * H], F32)
            nc.sync.dma_start(out=wih_a[:], in_=wiht[0:128, :])
            wih_b = sb.tile([128, 3 * H], F32)
            nc.sync.dma_start(out=wih_b[:], in_=wiht[128:256, :])
            wih_c = sb.tile([1, 3 * H], F32)
            nc.sync.dma_start(out=wih_c[:], in_=wiht[256:257, :])
            ones_t = sb.tile([1, CELLS1], F32)
            nc.vector.memset(ones_t[:], 1.0)
            ident = sb.tile([128, 128], F32)
            make_identity(nc, ident[:])

            # weighted gather-sum, produced directly transposed:
            #   out = G.T @ wmask  ->  (E-tile partitions, 4 cells)
            embTr = [sb.tile([128, CELLS1], F32, tag=f"embTr{k}", name=f"embTr{k}") for k in range(2)]
            for g in range(CHUNKS1 // GRP):
                G = gp.tile([128, GRP, E], F32, tag="G")
                for j in range(GRP):
                    c = g * GRP + j
                    nc.gpsimd.indirect_dma_start(
                        out=G[:, j, :],
                        out_offset=None,
                        in_=emb_w[:],
                        in_offset=bass.IndirectOffsetOnAxis(ap=idx_t[:, c : c + 1], axis=0),
                    )
                for j in range(GRP):
                    c = g * GRP + j
                    mp = pse.tile([128, 8], F32, tag="mps", bufs=4)
                    for k in range(2):
                        nc.tensor.matmul(
                            mp[:, 4 * k : 4 * k + 4],
                            G[:, j, 128 * k : 128 * (k + 1)],
                            wm_t[:, 4 * c : 4 * c + 4],
                            start=True,
                            stop=True,
                        )
                    for k in range(2):
                        nc.vector.tensor_copy(
                            embTr[k][:, 4 * c : 4 * c + 4], mp[:, 4 * k : 4 * k + 4]
                        )
            embT = [sb.tile([128, CELLS1], F32, tag=f"embT{k}", name=f"embT{k}") for k in range(2)]
            for k in range(2):
                nc.scalar.activation(embT[k][:], embTr[k][:], AF.Tanh)

            gi_sb = sb.tile([128, 2 * 3 * H], F32)
            for m in range(2):
                ms = slice(128 * m, 128 * (m + 1))
                gps = psg.tile([128, 3 * H], F32, tag="gips")
                for ch in range(3):
                    cs = slice(512 * ch, 512 * (ch + 1))
                    nc.tensor.matmul(gps[:, cs], embT[0][:, ms], wih_a[:, cs], start=True, stop=False)
                    nc.tensor.matmul(gps[:, cs], embT[1][:, ms], wih_b[:, cs], start=False, stop=False)
                    nc.tensor.matmul(gps[:, cs], ones_t[:, ms], wih_c[:, cs], start=False, stop=True)
                nc.vector.tensor_copy(gi_sb[:, 1536 * m : 1536 * (m + 1)], gps[:])
            nc.sync.dma_start(
                out=gi_out[:].rearrange("(m p) j -> p m j", m=2),
                in_=gi_sb[:].rearrange("p (m j) -> p m j", m=2),
            )
    _legalize_waits(nc)
    return nc


# ------------------------------------------------------------------ K2
def build_k2(gru_terms=3):
    nc = bass.Bass("TRN2", target_bir_lowering=False)
    gi_seq = nc.declare_dram_parameter("gi_seq", [S, B, 3 * H], F32, isOutput=False)
    zt1 = nc.declare_dram_parameter("zt1", [L + 1, B], F32, isOutput=False)
    wl2ht = nc.declare_dram_parameter("wl2ht", [L + 1, H], F32, isOutput=False)
    whht_hi = nc.declare_dram_parameter("whht_hi", [H, 3 * H], F32R, isOutput=False)
    whht_lo = nc.declare_dram_parameter("whht_lo", [H, 3 * H], F32R, isOutput=False)
    woutt = nc.declare_dram_parameter("woutt", [H, VSL], F32R, isOutput=False)
    bout_bc = nc.declare_dram_parameter("bout_bc", [128, VSL], F32, isOutput=False)
    valid_bc = nc.declare_dram_parameter("valid_bc", [128, CELLS], F32, isOutput=False)
    bhh_n = nc.declare_dram_parameter("bhh_n", [1, H], F32R, isOutput=False)
    p_out = nc.declare_dram_parameter("p_out", [CELLS, VSL], F32, isOutput=True)
    outt_out = nc.declare_dram_parameter("outt_out", [H, CELLS], F32, isOutput=True)

    KT = H // 128  # 4
    with TileContext(nc) as tc:
        with tc.tile_pool(name="wsb", bufs=1) as wsb, \
             tc.tile_pool(name="gip", bufs=2) as gip, \
             tc.tile_pool(name="ewp", bufs=1) as ewp, \
             tc.tile_pool(name="hsp", bufs=2) as hsp, \
             tc.tile_pool(name="lgp", bufs=2) as lgp, \
             tc.tile_pool(name="lorp", bufs=2) as lorp:
            whi = [wsb.tile([128, 3 * H], F32R, tag=f"whi{k}", name=f"whi{k}") for k in range(KT)]
            wlo = [wsb.tile([128, 3 * H], F32R, tag=f"wlo{k}", name=f"wlo{k}") for k in range(KT)] \
                if gru_terms == 3 else None
            wo = [wsb.tile([128, VSL], F32R, tag=f"wo{k}", name=f"wo{k}") for k in range(KT)]
            for k in range(KT):
                ks = slice(128 * k, 128 * (k + 1))
                nc.sync.dma_start(out=whi[k][:], in_=whht_hi[ks, :])
                if gru_terms == 3:
                    nc.sync.dma_start(out=wlo[k][:], in_=whht_lo[ks, :])
                nc.sync.dma_start(out=wo[k][:], in_=woutt[ks, :])
            bout_t = wsb.tile([128, VSL], F32)
            nc.sync.dma_start(out=bout_t[:], in_=bout_bc[:])
            valid_t = wsb.tile([128, CELLS], F32)
            nc.sync.dma_start(out=valid_t[:], in_=valid_bc[:])
            zt_a = wsb.tile([128, B], F32)
            nc.sync.dma_start(out=zt_a[:], in_=zt1[0:128, :])
            zt_b = wsb.tile([1, B], F32)
            nc.sync.dma_start(out=zt_b[:], in_=zt1[128:129, :])
            wl_a = wsb.tile([128, H], F32)
            nc.sync.dma_start(out=wl_a[:], in_=wl2ht[0:128, :])
            wl_b = wsb.tile([1, H], F32)
            nc.sync.dma_start(out=wl_b[:], in_=wl2ht[128:129, :])
            ident = wsb.tile([128, 128], F32)
            make_identity(nc, ident[:])
            bhh_t = wsb.tile([1, H], F32R)
            nc.sync.dma_start(out=bhh_t[:], in_=bhh_n[:])
            ones_f = wsb.tile([1, B], F32)
            nc.vector.memset(ones_f[:], 1.0)
            ones_r = wsb.tile([1, B], F32R)
            nc.vector.tensor_copy(ones_r[:], ones_f[:])
            outT = [wsb.tile([128, CELLS], F32, tag=f"outT{k}", name=f"outT{k}") for k in range(KT)]

            with tc.tile_pool(name="psg", bufs=2, space="PSUM") as psg, \
                 tc.tile_pool(name="psh", bufs=2, space="PSUM") as psh:
                # h0 = z @ W_l2h.T + b_l2h (fp32 exact)
                h0ps = psg.tile([B, H], F32, tag="ghr")
                nc.tensor.matmul(h0ps[:], zt_a[:], wl_a[:], start=True, stop=False)
                nc.tensor.matmul(h0ps[:], zt_b[:], wl_b[:], start=False, stop=True)
                h = hsp.tile([B, H], F32, tag="h")
                nc.vector.tensor_copy(h[:], h0ps[:])

                def transpose_split(h_cur, s):
                    his = [None] * KT
                    los = [None] * KT
                    for k in range(KT):
                        tp = psh.tile([128, B], F32, tag="tp")
                        nc.tensor.transpose(
                            out=tp[:], in_=h_cur[:, 128 * k : 128 * (k + 1)],
                            identity=ident[0:B, 0:B],
                        )
                        hi = hsp.tile([128, B], F32R, tag=f"hi{k}")
                        nc.vector.tensor_copy(hi[:], tp[:])
                        his[k] = hi
                        if gru_terms == 3:
                            lo = hsp.tile([128, B], F32R, tag=f"lo{k}")
                            nc.vector.tensor_tensor(
                                out=lo[:], in0=tp[:], in1=hi[:], op=OP.subtract
                            )
                            los[k] = lo
                        if s >= 0:
                            # outputsT[k] columns s*B..s*B+B = h_s.T * valid
                            o_ap = outT[k][:, B * s : B * (s + 1)]
                            nc.vector.tensor_tensor(
                                out=o_ap, in0=tp[:], in1=valid_t[:, B * s : B * (s + 1)],
                                op=OP.mult,
                            )
                    return his, los

                hT_hi, hT_lo = transpose_split(h, -1)

                def gate_mms(dst, cs, hi, lo, close=True):
                    for k in range(KT):
                        fin = close and k == KT - 1 and gru_terms != 3
                        nc.tensor.matmul(dst[:], hi[k][:], whi[k][:, cs],
                                         start=(k == 0), stop=fin)
                        if gru_terms == 3:
                            nc.tensor.matmul(dst[:], hi[k][:], wlo[k][:, cs],
                                             start=False, stop=False)
                            nc.tensor.matmul(dst[:], lo[k][:], whi[k][:, cs],
                                             start=False, stop=(close and k == KT - 1))

                for s in range(S):
                    gi_t = gip.tile([B, 3 * H], F32, tag="gi")
                    nc.sync.dma_start(out=gi_t[:], in_=gi_seq[s, :, :])
                    # separate PSUM tiles per gate chunk; emission order r, n, z
                    # so the sigmoid(r)/tanh(n) chain overlaps later matmuls
                    gh_r = psg.tile([B, H], F32, tag="ghr")
                    gh_n = psg.tile([B, H], F32, tag="ghn")
                    gh_z = psg.tile([B, H], F32, tag="ghz")
                    gate_mms(gh_r, slice(0, H), hT_hi, hT_lo)
                    # n-gate: gh_n must include b_hh_n (scaled by r later)
                    gate_mms(gh_n, slice(2 * H, 3 * H), hT_hi, hT_lo, close=False)
                    nc.tensor.matmul(gh_n[:], ones_r[:], bhh_t[:], start=False, stop=True)
                    t_r = ewp.tile([B, H], F32, tag="t_r")
                    nc.vector.tensor_tensor(out=t_r[:], in0=gh_r[:], in1=gi_t[:, 0:H], op=OP.add)
                    r_t = ewp.tile([B, H], F32, tag="r_t")
                    nc.scalar.activation(r_t[:], t_r[:], AF.Sigmoid)
                    gate_mms(gh_z, slice(H, 2 * H), hT_hi, hT_lo)
                    tn = ewp.tile([B, H], F32, tag="tn")
                    nc.vector.tensor_tensor(out=tn[:], in0=gh_n[:], in1=r_t[:], op=OP.mult)
                    tn2 = ewp.tile([B, H], F32, tag="tn2")
                    nc.vector.tensor_tensor(out=tn2[:], in0=tn[:], in1=gi_t[:, 2 * H : 3 * H],
                                            op=OP.add)
                    n_t = ewp.tile([B, H], F32, tag="n")
                    nc.scalar.activation(n_t[:], tn2[:], AF.Tanh)
                    t_z = ewp.tile([B, H], F32, tag="t_z")
                    nc.vector.tensor_tensor(out=t_z[:], in0=gh_z[:], in1=gi_t[:, H : 2 * H],
                                            op=OP.add)
                    z_t = ewp.tile([B, H], F32, tag="z_t")
                    nc.scalar.activation(z_t[:], t_z[:], AF.Sigmoid)
                    d_t = ewp.tile([B, H], F32, tag="d")
                    nc.vector.tensor_tensor(out=d_t[:], in0=h[:], in1=n_t[:], op=OP.subtract)
                    e_t = ewp.tile([B, H], F32, tag="e")
                    nc.vector.tensor_tensor(out=e_t[:], in0=d_t[:], in1=z_t[:], op=OP.mult)
                    h = hsp.tile([B, H], F32, tag="h")
                    nc.vector.tensor_tensor(out=h[:], in0=e_t[:], in1=n_t[:], op=OP.add)
                    hT_hi, hT_lo = transpose_split(h, s)

            # logits + sigmoid over the vocab slice
            with tc.tile_pool(name="psl", bufs=4, space="PSUM") as psl:
                NCH = 5
                CW = VSL // NCH
                for m in range(CELLS // 128):
                    ms = slice(128 * m, 128 * (m + 1))
                    lhs = []
                    for k in range(KT):
                        lr = lorp.tile([128, 128], F32R, tag=f"lr{k}")
                        nc.vector.tensor_copy(lr[:], outT[k][:, ms])
                        lhs.append(lr)
                    for c in range(NCH):
                        cs = slice(CW * c, CW * (c + 1))
                        lp = psl.tile([128, CW], F32, tag="lp")
                        for k in range(KT):
                            nc.tensor.matmul(lp[:], lhs[k][:], wo[k][:, cs],
                                             start=(k == 0), stop=(k == KT - 1))
                        lg = lgp.tile([128, CW], F32, tag="lg")
                        nc.vector.tensor_tensor(out=lg[:], in0=lp[:], in1=bout_t[:, cs], op=OP.add)
                        pt = lgp.tile([128, CW], F32, tag="pt")
                        nc.scalar.activation(pt[:], lg[:], AF.Sigmoid)
                        nc.sync.dma_start(out=p_out[ms, cs], in_=pt[:])
                for k in range(KT):
                    nc.sync.dma_start(out=outt_out[128 * k : 128 * (k + 1), :], in_=outT[k][:])
    _legalize_waits(nc)
    return nc


_K1 = None
_K2 = None
DEBUG = {}
TRACE = False
LAST_HW_NS = None


def _get_kernels():
    global _K1, _K2
    if _K1 is None:
        _K1 = build_k1()
    if _K2 is None:
        _K2 = build_k2(gru_terms=3)
    return _K1, _K2


# ------------------------------------------------------------------ host glue
def _dedup_weights(iseq):
    """multihot scatter-set semantics: weight 1.0 for the first occurrence of
    each index in a cell, 0 for repeats and for PAD_IDX."""
    idx = np.asarray(iseq, dtype=np.int64).reshape(B * S, MV)
    w = np.ones_like(idx, dtype=np.float32)
    order = np.argsort(idx, axis=1, kind="stable")
    sorted_vals = np.take_along_axis(idx, order, axis=1)
    dup_sorted = np.zeros_like(sorted_vals, dtype=bool)
    dup_sorted[:, 1:] = sorted_vals[:, 1:] == sorted_vals[:, :-1]
    dup = np.zeros_like(dup_sorted)
    np.put_along_axis(dup, order, dup_sorted, axis=1)
    w[dup] = 0.0
    w[idx == PAD_IDX] = 0.0
    return idx.reshape(B, S, MV), w.reshape(B, S, MV)


def kernel(**inputs):
    z = np.asarray(inputs["z"], np.float32)
    iseq = np.asarray(inputs["input_sequence"])
    length = np.asarray(inputs["length"]).astype(np.int64)
    emb_w = np.ascontiguousarray(np.asarray(inputs["embedding_weight"], np.float32))
    W_l2h = np.asarray(inputs["W_l2h"], np.float32)
    b_l2h = np.asarray(inputs["b_l2h"], np.float32)
    W_ih = np.asarray(inputs["W_ih"], np.float32)
    W_hh = np.asarray(inputs["W_hh"], np.float32)
    b_ih = np.asarray(inputs["b_ih"], np.float32)
    b_hh = np.asarray(inputs["b_hh"], np.float32)
    W_out = np.asarray(inputs["W_out"], np.float32)
    b_out = np.asarray(inputs["b_out"], np.float32)

    k1, k2 = _get_kernels()

    # ---- K1
    idx_full, w_full = _dedup_weights(iseq)
    bias_row = np.concatenate([(b_ih + b_hh)[: 2 * H], b_ih[2 * H :]])
    wiht = np.concatenate([W_ih.T, bias_row[None, :]], axis=0).astype(np.float32)
    in_maps1 = []
    for c in range(NC):
        bs = slice(c * BL, (c + 1) * BL)
        idx_c = np.zeros((CELLS1, SLOT), np.int64)
        w_c = np.zeros((CELLS1, SLOT), np.float32)
        idx_c[:, :MV] = idx_full[bs].reshape(CELLS1, MV)
        w_c[:, :MV] = w_full[bs].reshape(CELLS1, MV)
        rows = idx_c.reshape(CHUNKS1, 4 * SLOT)      # (64,128): chunk x row
        wrow = w_c.reshape(CHUNKS1, 4 * SLOT)
        idx_in = np.ascontiguousarray(rows.T).astype(np.int32)  # (128, 64)
        wm = np.zeros((128, CHUNKS1, 4), np.float32)
        for jcell in range(4):
            wm[jcell * SLOT : (jcell + 1) * SLOT, :, jcell] = (
                wrow[:, jcell * SLOT : (jcell + 1) * SLOT].T
            )
        in_maps1.append({
            "emb_w": emb_w,
            "idx": idx_in,
            "wmask": np.ascontiguousarray(wm.reshape(128, CHUNKS1 * 4)),
            "wiht": wiht,
        })
    r1 = run_bass_kernel_spmd(k1, in_maps1, core_ids=list(range(NC)), trace=TRACE)
    gi_all = np.concatenate(
        [r1.results[c]["gi_out"].reshape(BL, S, 3 * H) for c in range(NC)], axis=0
    )  # (B, S, 3H)
    gi_seq = np.ascontiguousarray(gi_all.transpose(1, 0, 2))  # (S, B, 3H)

    # ---- K2
    zt1 = np.concatenate([z.T, np.ones((1, B), np.float32)], axis=0)
    wl2ht = np.concatenate([W_l2h.T, b_l2h[None, :]], axis=0).astype(np.float32)
    whhT = np.ascontiguousarray(W_hh.T)
    whh_hi = whhT.astype(ml_dtypes.bfloat16).astype(np.float32)
    whh_lo = np.ascontiguousarray(whhT - whh_hi)
    valid = (np.arange(S)[None, :] < length[:, None]).astype(np.float32)  # (B,S)
    # cell order is s-major: col = s*B + b -> valid.T flattened
    valid_cells = np.ascontiguousarray(valid.T).reshape(1, CELLS)
    valid_bc = np.broadcast_to(valid_cells, (128, CELLS)).astype(np.float32).copy()
    woutT = np.ascontiguousarray(W_out.T)
    in_maps2 = []
    for c in range(NC):
        vs = slice(c * VSL, (c + 1) * VSL)
        in_maps2.append({
            "gi_seq": gi_seq,
            "zt1": zt1,
            "wl2ht": wl2ht,
            "whht_hi": whh_hi,
            "whht_lo": whh_lo,
            "woutt": np.ascontiguousarray(woutT[:, vs]),
            "bout_bc": np.broadcast_to(b_out[vs][None, :], (128, VSL)).astype(np.float32).copy(),
            "valid_bc": valid_bc,
            "bhh_n": b_hh[2 * H :][None, :].astype(np.float32),
        })
    r2 = run_bass_kernel_spmd(k2, in_maps2, core_ids=list(range(NC)), trace=TRACE)

    # ---- assemble outputs (cells are s-major: cell = s*B + b)
    p_cells = np.concatenate([r2.results[c]["p_out"] for c in range(NC)], axis=1)  # (2048, V)
    p_output = np.ascontiguousarray(p_cells.reshape(S, B, V).transpose(1, 0, 2))

    m_output = np.arange(S)[None, :] < length[:, None]

    outputs = r2.results[0]["outt_out"].T.astype(np.float64)  # (2048, H)
    global DEBUG, LAST_HW_NS
    DEBUG = {"outt": outputs, "p_cells": p_cells, "r1": r1, "r2": r2}
    if r1.exec_time_ns or r2.exec_time_ns:
        LAST_HW_NS = int((r1.exec_time_ns or 0) + (r2.exec_time_ns or 0))
    NCAND = 128
    sub = p_cells[:, 3:]
    cand = np.argpartition(-sub, NCAND, axis=1)[:, :NCAND] + 3
    Wc = W_out.astype(np.float64)[cand]
    logit64 = np.einsum("ch,cgh->cg", outputs, Wc) + b_out.astype(np.float64)[cand]
    ordk = np.lexsort((cand, -logit64), axis=1)[:, :MV]
    top_i = np.take_along_axis(cand, ordk, axis=1).astype(np.int32)
    top_v = np.take_along_axis(logit64, ordk, axis=1)
    s_cells = np.where(top_v > 0.0, top_i, np.int32(PAD_IDX)).astype(np.int32)
    s_output = np.ascontiguousarray(s_cells.reshape(S, B, MV).transpose(1, 0, 2))

    return (p_output.astype(np.float32), s_output, m_output)


# revision 21
# speedup vs baseline: 1.1105x; 1.1105x over previous
"""Trainium2 Bass kernel for nn_Decoder (multihot-embedding -> GRU -> vocab logits).

Strategy (8 NeuronCores):
  K1: batch-parallel embedding stage. Each core handles B/8 = 4 batch rows:
      indirect-DMA gathers of embedding rows (host-deduped multihot weights),
      weighted-sum via small PE matmuls, tanh, then the (cells, 3H) input-gate
      matmul gi = emb @ W_ih.T + (b_ih + b_hh) in full fp32.
  K2: replicated GRU (all cores run the identical B=32 recurrence) + vocab-
      sharded logits/sigmoid (each core computes its V/8 = 2500 W_out slice).
      The GRU's W_hh @ h matmul runs as a 3-term fp32r hi/lo split (bf16-based
      split of W on the host) giving ~fp32 accuracy at 1 cyc/row. The logits
      matmul runs in bf16 (p_output tolerance is orders of magnitude above its
      error; exact ordering is restored on the host).
  Host: shards/reshapes inputs, dedups multihot indices, assembles p_output,
      computes m_output from `length`, and finalizes s_output by top-k over the
      kernel's p_output with an fp64 rerank of candidate logits (built from the
      kernel's fp32-accurate GRU outputs).
"""
import sys

sys.path.insert(0, "/opt/trn_rl_repo")
import numpy as np
import ml_dtypes

import concourse.bass as bass
import concourse.mybir as mybir
from concourse.tile import TileContext
from concourse.masks import make_identity
from concourse.bass_utils import run_bass_kernel_spmd

import bass_rust

B, S, MV = 32, 64, 30
V, E, H, L = 20000, 256, 512, 128
PAD_IDX = 0
NC = 8
BL = B // NC             # batches per core in K1 (4)
CELLS1 = BL * S          # cells per core in K1 (256)
SLOT = 32                # padded gather slots per cell
CHUNKS1 = CELLS1 * SLOT // 128  # 64 gather chunks of 128 rows
VSL = V // NC            # vocab slice per core in K2 (2500)
CELLS = B * S            # 2048; K2 cell order is s-major: cell = s*B + b
F32 = mybir.dt.float32
F32R = mybir.dt.float32r
BF16 = mybir.dt.bfloat16
I32 = mybir.dt.int32
AF = mybir.ActivationFunctionType
OP = mybir.AluOpType


def _legalize_waits(nc, max_waits=1):
    """This walrus build encodes one sync-wait per 64B instruction and rejects
    more; hoist extra waits onto per-engine NoOps inserted just before."""
    n = 0
    for fn in nc.m.functions:
        for bb in fn.blocks:
            out, changed = [], False
            for inst in bb.instructions:
                si = inst.sync_info
                waits = list(si.on_wait) if si is not None and si.on_wait else []
                if len(waits) > max_waits:
                    for j, w in enumerate(waits[:-max_waits]):
                        nop = bass_rust.InstNoOp(name=f"{inst.name}-lw{j}", engine=inst.engine)
                        nop.sync_info = mybir.SyncInfo(on_wait=[w], on_update=[])
                        out.append(nop)
                    inst.sync_info = mybir.SyncInfo(
                        on_wait=waits[-max_waits:], on_update=list(si.on_update or [])
                    )
                    n += 1
                    changed = True
                out.append(inst)
            if changed:
                bb.instructions = out
    return n


# ------------------------------------------------------------------ K1
def build_k1():
    nc = bass.Bass("TRN2", target_bir_lowering=False)
    emb_w = nc.declare_dram_parameter("emb_w", [V, E], F32, isOutput=False)
    idx = nc.declare_dram_parameter("idx", [128, CHUNKS1], I32, isOutput=False)
    wmask = nc.declare_dram_parameter("wmask", [128, CHUNKS1 * 4], F32, isOutput=False)
    wiht = nc.declare_dram_parameter("wiht", [E + 1, 3 * H], F32, isOutput=False)
    gi_out = nc.declare_dram_parameter("gi_out", [CELLS1, 3 * H], F32, isOutput=True)

    GRP = 8  # gather chunks per G tile
    with TileContext(nc) as tc:
        with tc.tile_pool(name="sb", bufs=1) as sb, \
             tc.tile_pool(name="gp", bufs=4) as gp, \
             tc.tile_pool(name="pse", bufs=1, space="PSUM") as pse, \
             tc.tile_pool(name="psg", bufs=1, space="PSUM") as psg, \
             tc.tile_pool(name="pst", bufs=2, space="PSUM") as pst:
            idx_t = sb.tile([128, CHUNKS1], I32)
            nc.sync.dma_start(out=idx_t[:], in_=idx[:])
            wm_t = sb.tile([128, CHUNKS1 * 4], F32)
            nc.sync.dma_start(out=wm_t[:], in_=wmask[:])
            wih_a = sb.tile([128, 3 * H], F32)
            nc.sync.dma_start(out=wih_a[:], in_=wiht[0:128, :])
            wih_b = sb.tile([128, 3 * H], F32)
            nc.sync.dma_start(out=wih_b[:], in_=wiht[128:256, :])
            wih_c = sb.tile([1, 3 * H], F32)
            nc.sync.dma_start(out=wih_c[:], in_=wiht[256:257, :])
            ones_t = sb.tile([1, CELLS1], F32)
            nc.vector.memset(ones_t[:], 1.0)
            ident = sb.tile([128, 128], F32)
            make_identity(nc, ident[:])

            # weighted gather-sum, produced directly transposed:
            #   out = G.T @ wmask  ->  (E-tile partitions, 4 cells)
            embTr = [sb.tile([128, CELLS1], F32, tag=f"embTr{k}", name=f"embTr{k}") for k in range(2)]
            for g in range(CHUNKS1 // GRP):
                G = gp.tile([128, GRP, E], F32, tag="G")
                for j in range(GRP):
                    c = g * GRP + j
                    nc.gpsimd.indirect_dma_start(
                        out=G[:, j, :],
                        out_offset=None,
                        in_=emb_w[:],
                        in_offset=bass.IndirectOffsetOnAxis(ap=idx_t[:, c : c + 1], axis=0),
                    )
                for j in range(GRP):
                    c = g * GRP + j
                    mp = pse.tile([128, 8], F32, tag="mps", bufs=4)
                    for k in range(2):
                        nc.tensor.matmul(
                            mp[:, 4 * k : 4 * k + 4],
                            G[:, j, 128 * k : 128 * (k + 1)],
                            wm_t[:, 4 * c : 4 * c + 4],
                            start=True,
                            stop=True,
                        )
                    for k in range(2):
                        nc.vector.tensor_copy(
                            embTr[k][:, 4 * c : 4 * c + 4], mp[:, 4 * k : 4 * k + 4]
                        )
            embT = [sb.tile([128, CELLS1], F32, tag=f"embT{k}", name=f"embT{k}") for k in range(2)]
            for k in range(2):
                nc.scalar.activation(embT[k][:], embTr[k][:], AF.Tanh)

            gi_sb = sb.tile([128, 2 * 3 * H], F32)
            for m in range(2):
                ms = slice(128 * m, 128 * (m + 1))
                gps = psg.tile([128, 3 * H], F32, tag="gips")
                for ch in range(3):
                    cs = slice(512 * ch, 512 * (ch + 1))
                    nc.tensor.matmul(gps[:, cs], embT[0][:, ms], wih_a[:, cs], start=True, stop=False)
                    nc.tensor.matmul(gps[:, cs], embT[1][:, ms], wih_b[:, cs], start=False, stop=False)
                    nc.tensor.matmul(gps[:, cs], ones_t[:, ms], wih_c[:, cs], start=False, stop=True)
                nc.vector.tensor_copy(gi_sb[:, 1536 * m : 1536 * (m + 1)], gps[:])
            nc.sync.dma_start(
                out=gi_out[:].rearrange("(m p) j -> p m j", m=2),
                in_=gi_sb[:].rearrange("p (m j) -> p m j", m=2),
            )
    _legalize_waits(nc)
    return nc


# ------------------------------------------------------------------ K2
def build_k2(gru_terms=3):
    nc = bass.Bass("TRN2", target_bir_lowering=False)
    gi_seq = nc.declare_dram_parameter("gi_seq", [S, B, 3 * H], F32, isOutput=False)
    zt1 = nc.declare_dram_parameter("zt1", [L + 1, B], F32, isOutput=False)
    wl2ht = nc.declare_dram_parameter("wl2ht", [L + 1, H], F32, isOutput=False)
    whht_hi = nc.declare_dram_parameter("whht_hi", [H, 3 * H], F32R, isOutput=False)
    whht_lo = nc.declare_dram_parameter("whht_lo", [H, 3 * H], F32R, isOutput=False)
    woutt = nc.declare_dram_parameter("woutt", [H, VSL], F32R, isOutput=False)
    bout_bc = nc.declare_dram_parameter("bout_bc", [128, VSL], F32, isOutput=False)
    valid_bc = nc.declare_dram_parameter("valid_bc", [128, CELLS], F32, isOutput=False)
    bhh_n = nc.declare_dram_parameter("bhh_n", [1, H], F32R, isOutput=False)
    p_out = nc.declare_dram_parameter("p_out", [CELLS, VSL], F32, isOutput=True)
    outt_out = nc.declare_dram_parameter("outt_out", [H, CELLS], F32, isOutput=True)

    KT = H // 128  # 4
    with TileContext(nc) as tc:
        with tc.tile_pool(name="wsb", bufs=1) as wsb, \
             tc.tile_pool(name="gip", bufs=2) as gip, \
             tc.tile_pool(name="ewp", bufs=1) as ewp, \
             tc.tile_pool(name="hsp", bufs=2) as hsp, \
             tc.tile_pool(name="lgp", bufs=2) as lgp, \
             tc.tile_pool(name="lorp", bufs=2) as lorp:
            whi = [wsb.tile([128, 3 * H], F32R, tag=f"whi{k}", name=f"whi{k}") for k in range(KT)]
            wlo = [wsb.tile([128, 3 * H], F32R, tag=f"wlo{k}", name=f"wlo{k}") for k in range(KT)] \
                if gru_terms == 3 else None
            wo = [wsb.tile([128, VSL], F32R, tag=f"wo{k}", name=f"wo{k}") for k in range(KT)]
            for k in range(KT):
                ks = slice(128 * k, 128 * (k + 1))
                nc.sync.dma_start(out=whi[k][:], in_=whht_hi[ks, :])
                if gru_terms == 3:
                    nc.sync.dma_start(out=wlo[k][:], in_=whht_lo[ks, :])
                nc.sync.dma_start(out=wo[k][:], in_=woutt[ks, :])
            bout_t = wsb.tile([128, VSL], F32)
            nc.sync.dma_start(out=bout_t[:], in_=bout_bc[:])
            valid_t = wsb.tile([128, CELLS], F32)
            nc.sync.dma_start(out=valid_t[:], in_=valid_bc[:])
            zt_a = wsb.tile([128, B], F32)
            nc.sync.dma_start(out=zt_a[:], in_=zt1[0:128, :])
            zt_b = wsb.tile([1, B], F32)
            nc.sync.dma_start(out=zt_b[:], in_=zt1[128:129, :])
            wl_a = wsb.tile([128, H], F32)
            nc.sync.dma_start(out=wl_a[:], in_=wl2ht[0:128, :])
            wl_b = wsb.tile([1, H], F32)
            nc.sync.dma_start(out=wl_b[:], in_=wl2ht[128:129, :])
            ident = wsb.tile([128, 128], F32)
            make_identity(nc, ident[:])
            bhh_t = wsb.tile([1, H], F32R)
            nc.sync.dma_start(out=bhh_t[:], in_=bhh_n[:])
            ones_f = wsb.tile([1, B], F32)
            nc.vector.memset(ones_f[:], 1.0)
            ones_r = wsb.tile([1, B], F32R)
            nc.vector.tensor_copy(ones_r[:], ones_f[:])
            NM = CELLS // 128  # 16 logits m-tiles; m-tile m = steps 4m..4m+3
            outTs = [[wsb.tile([128, 128], F32, tag=f"oT{k}_{m}", name=f"oT{k}_{m}")
                      for m in range(NM)] for k in range(KT)]

            with tc.tile_pool(name="psg", bufs=1, space="PSUM") as psg, \
                 tc.tile_pool(name="psh", bufs=1, space="PSUM") as psh, \
                 tc.tile_pool(name="psl", bufs=2, space="PSUM") as psl:
                NCH = 5
                CW = VSL // NCH

                def emit_logits_mtile(m):
                    lhs = []
                    for k in range(KT):
                        lr = lorp.tile([128, 128], F32R, tag=f"lr{k}")
                        nc.vector.tensor_copy(lr[:], outTs[k][m][:])
                        lhs.append(lr)
                    ms = slice(128 * m, 128 * (m + 1))
                    for c in range(NCH):
                        cs = slice(CW * c, CW * (c + 1))
                        lp = psl.tile([128, CW], F32, tag="lp")
                        for k in range(KT):
                            nc.tensor.matmul(lp[:], lhs[k][:], wo[k][:, cs],
                                             start=(k == 0), stop=(k == KT - 1))
                        lg = lgp.tile([128, CW], F32, tag="lg")
                        nc.vector.tensor_tensor(out=lg[:], in0=lp[:], in1=bout_t[:, cs], op=OP.add)
                        pt = lgp.tile([128, CW], F32, tag="pt")
                        nc.scalar.activation(pt[:], lg[:], AF.Sigmoid)
                        nc.sync.dma_start(out=p_out[ms, cs], in_=pt[:])
                # h0 = z @ W_l2h.T + b_l2h (fp32 exact)
                h0ps = psg.tile([B, H], F32, tag="ghr")
                nc.tensor.matmul(h0ps[:], zt_a[:], wl_a[:], start=True, stop=False)
                nc.tensor.matmul(h0ps[:], zt_b[:], wl_b[:], start=False, stop=True)
                h = hsp.tile([B, H], F32, tag="h")
                nc.vector.tensor_copy(h[:], h0ps[:])

                def transpose_split(h_cur, s):
                    his = [None] * KT
                    los = [None] * KT
                    tps = [None] * KT
                    for k in range(KT):
                        tp = psh.tile([128, B], F32, tag="tp", bufs=2)
                        nc.tensor.transpose(
                            out=tp[:], in_=h_cur[:, 128 * k : 128 * (k + 1)],
                            identity=ident[0:B, 0:B],
                        )
                        tps[k] = tp
                        hi = hsp.tile([128, B], F32R, tag=f"hi{k}")
                        nc.vector.tensor_copy(hi[:], tp[:])
                        his[k] = hi
                        if gru_terms == 3:
                            lo = hsp.tile([128, B], F32R, tag=f"lo{k}")
                            nc.vector.tensor_tensor(
                                out=lo[:], in0=tp[:], in1=hi[:], op=OP.subtract
                            )
                            los[k] = lo
                    return his, los

                hT_hi, hT_lo = transpose_split(h, -1)

                def gate_mms(dst, cs, hi, lo, close=True):
                    for k in range(KT):
                        fin = close and k == KT - 1 and gru_terms != 3
                        nc.tensor.matmul(dst[:], hi[k][:], whi[k][:, cs],
                                         start=(k == 0), stop=fin)
                        if gru_terms == 3:
                            nc.tensor.matmul(dst[:], hi[k][:], wlo[k][:, cs],
                                             start=False, stop=False)
                            nc.tensor.matmul(dst[:], lo[k][:], whi[k][:, cs],
                                             start=False, stop=(close and k == KT - 1))

                for s in range(S):
                    gi_t = gip.tile([B, 3 * H], F32, tag="gi")
                    nc.sync.dma_start(out=gi_t[:], in_=gi_seq[s, :, :])
                    # separate PSUM tiles per gate chunk; emission order r, n, z
                    # so the sigmoid(r)/tanh(n) chain overlaps later matmuls
                    gh_r = psg.tile([B, H], F32, tag="ghr")
                    gh_n = psg.tile([B, H], F32, tag="ghn")
                    gh_z = psg.tile([B, H], F32, tag="ghz")
                    gate_mms(gh_r, slice(0, H), hT_hi, hT_lo)
                    # n-gate: gh_n must include b_hh_n (scaled by r later)
                    gate_mms(gh_n, slice(2 * H, 3 * H), hT_hi, hT_lo, close=False)
                    nc.tensor.matmul(gh_n[:], ones_r[:], bhh_t[:], start=False, stop=True)
                    t_r = ewp.tile([B, H], F32, tag="t_r")
                    nc.vector.tensor_tensor(out=t_r[:], in0=gh_r[:], in1=gi_t[:, 0:H], op=OP.add)
                    r_t = ewp.tile([B, H], F32, tag="r_t")
                    nc.scalar.activation(r_t[:], t_r[:], AF.Sigmoid)
                    gate_mms(gh_z, slice(H, 2 * H), hT_hi, hT_lo)
                    n_t = ewp.tile([B, H], F32, tag="n")
                    for k in range(KT):
                        ks = slice(128 * k, 128 * (k + 1))
                        tn = ewp.tile([B, 128], F32, tag=f"tn{k}")
                        nc.vector.tensor_tensor(out=tn[:], in0=gh_n[:, ks], in1=r_t[:, ks],
                                                op=OP.mult)
                        tn2 = ewp.tile([B, 128], F32, tag=f"tn2{k}")
                        nc.vector.tensor_tensor(out=tn2[:], in0=tn[:],
                                                in1=gi_t[:, 2 * H + 128 * k : 2 * H + 128 * (k + 1)],
                                                op=OP.add)
                        nc.scalar.activation(n_t[:, ks], tn2[:], AF.Tanh)
                    t_z = ewp.tile([B, H], F32, tag="t_z")
                    nc.vector.tensor_tensor(out=t_z[:], in0=gh_z[:], in1=gi_t[:, H : 2 * H],
                                            op=OP.add)
                    z_t = ewp.tile([B, H], F32, tag="z_t")
                    nc.scalar.activation(z_t[:], t_z[:], AF.Sigmoid)
                    # end-of-step tail chunked by H-128 so next-step matmuls
                    # (which consume hT k-tiles in order) start after slice 0
                    h_new = hsp.tile([B, H], F32, tag="h")
                    his = [None] * KT
                    los = [None] * KT
                    tps = [None] * KT
                    for k in range(KT):
                        ks = slice(128 * k, 128 * (k + 1))
                        d_t = ewp.tile([B, 128], F32, tag=f"d{k}")
                        nc.vector.tensor_tensor(out=d_t[:], in0=h[:, ks], in1=n_t[:, ks],
                                                op=OP.subtract)
                        e_t = ewp.tile([B, 128], F32, tag=f"e{k}")
                        nc.vector.tensor_tensor(out=e_t[:], in0=d_t[:], in1=z_t[:, ks],
                                                op=OP.mult)
                        nc.vector.tensor_tensor(out=h_new[:, ks], in0=e_t[:], in1=n_t[:, ks],
                                                op=OP.add)
                        tp = psh.tile([128, B], F32, tag="tp", bufs=2)
                        nc.tensor.transpose(out=tp[:], in_=h_new[:, ks],
                                            identity=ident[0:B, 0:B])
                        tps[k] = tp
                        hi = hsp.tile([128, B], F32R, tag=f"hi{k}")
                        nc.vector.tensor_copy(hi[:], tp[:])
                        his[k] = hi
                        lo = hsp.tile([128, B], F32R, tag=f"lo{k}")
                        nc.vector.tensor_tensor(out=lo[:], in0=tp[:], in1=hi[:], op=OP.subtract)
                        los[k] = lo
                    for k in range(KT):
                        o_ap = outTs[k][s // 4][:, B * (s % 4) : B * (s % 4 + 1)]
                        nc.vector.tensor_tensor(out=o_ap, in0=tps[k][:],
                                                in1=valid_t[:, B * s : B * (s + 1)], op=OP.mult)
                    h = h_new
                    hT_hi, hT_lo = his, los
                    if s % 4 == 3:
                        emit_logits_mtile(s // 4)

            # outputs.T to DRAM for the host fp64 rerank
            for k in range(KT):
                for m in range(CELLS // 128):
                    nc.sync.dma_start(
                        out=outt_out[128 * k : 128 * (k + 1), 128 * m : 128 * (m + 1)],
                        in_=outTs[k][m][:],
                    )
    _legalize_waits(nc)
    return nc


_K1 = None
_K2 = None
DEBUG = {}
TRACE = False
LAST_HW_NS = None


def _get_kernels():
    global _K1, _K2
    if _K1 is None:
        _K1 = build_k1()
    if _K2 is None:
        _K2 = build_k2(gru_terms=3)
    return _K1, _K2


# ------------------------------------------------------------------ host glue
def _dedup_weights(iseq):
    """multihot scatter-set semantics: weight 1.0 for the first occurrence of
    each index in a cell, 0 for repeats and for PAD_IDX."""
    idx = np.asarray(iseq, dtype=np.int64).reshape(B * S, MV)
    w = np.ones_like(idx, dtype=np.float32)
    order = np.argsort(idx, axis=1, kind="stable")
    sorted_vals = np.take_along_axis(idx, order, axis=1)
    dup_sorted = np.zeros_like(sorted_vals, dtype=bool)
    dup_sorted[:, 1:] = sorted_vals[:, 1:] == sorted_vals[:, :-1]
    dup = np.zeros_like(dup_sorted)
    np.put_along_axis(dup, order, dup_sorted, axis=1)
    w[dup] = 0.0
    w[idx == PAD_IDX] = 0.0
    return idx.reshape(B, S, MV), w.reshape(B, S, MV)


def kernel(**inputs):
    z = np.asarray(inputs["z"], np.float32)
    iseq = np.asarray(inputs["input_sequence"])
    length = np.asarray(inputs["length"]).astype(np.int64)
    emb_w = np.ascontiguousarray(np.asarray(inputs["embedding_weight"], np.float32))
    W_l2h = np.asarray(inputs["W_l2h"], np.float32)
    b_l2h = np.asarray(inputs["b_l2h"], np.float32)
    W_ih = np.asarray(inputs["W_ih"], np.float32)
    W_hh = np.asarray(inputs["W_hh"], np.float32)
    b_ih = np.asarray(inputs["b_ih"], np.float32)
    b_hh = np.asarray(inputs["b_hh"], np.float32)
    W_out = np.asarray(inputs["W_out"], np.float32)
    b_out = np.asarray(inputs["b_out"], np.float32)

    k1, k2 = _get_kernels()

    # ---- K1
    idx_full, w_full = _dedup_weights(iseq)
    bias_row = np.concatenate([(b_ih + b_hh)[: 2 * H], b_ih[2 * H :]])
    wiht = np.concatenate([W_ih.T, bias_row[None, :]], axis=0).astype(np.float32)
    in_maps1 = []
    for c in range(NC):
        bs = slice(c * BL, (c + 1) * BL)
        idx_c = np.zeros((CELLS1, SLOT), np.int64)
        w_c = np.zeros((CELLS1, SLOT), np.float32)
        idx_c[:, :MV] = idx_full[bs].reshape(CELLS1, MV)
        w_c[:, :MV] = w_full[bs].reshape(CELLS1, MV)
        rows = idx_c.reshape(CHUNKS1, 4 * SLOT)      # (64,128): chunk x row
        wrow = w_c.reshape(CHUNKS1, 4 * SLOT)
        idx_in = np.ascontiguousarray(rows.T).astype(np.int32)  # (128, 64)
        wm = np.zeros((128, CHUNKS1, 4), np.float32)
        for jcell in range(4):
            wm[jcell * SLOT : (jcell + 1) * SLOT, :, jcell] = (
                wrow[:, jcell * SLOT : (jcell + 1) * SLOT].T
            )
        in_maps1.append({
            "emb_w": emb_w,
            "idx": idx_in,
            "wmask": np.ascontiguousarray(wm.reshape(128, CHUNKS1 * 4)),
            "wiht": wiht,
        })
    r1 = run_bass_kernel_spmd(k1, in_maps1, core_ids=list(range(NC)), trace=TRACE)
    gi_all = np.concatenate(
        [r1.results[c]["gi_out"].reshape(BL, S, 3 * H) for c in range(NC)], axis=0
    )  # (B, S, 3H)
    gi_seq = np.ascontiguousarray(gi_all.transpose(1, 0, 2))  # (S, B, 3H)

    # ---- K2
    zt1 = np.concatenate([z.T, np.ones((1, B), np.float32)], axis=0)
    wl2ht = np.concatenate([W_l2h.T, b_l2h[None, :]], axis=0).astype(np.float32)
    whhT = np.ascontiguousarray(W_hh.T)
    whh_hi = whhT.astype(ml_dtypes.bfloat16).astype(np.float32)
    whh_lo = np.ascontiguousarray(whhT - whh_hi)
    valid = (np.arange(S)[None, :] < length[:, None]).astype(np.float32)  # (B,S)
    # cell order is s-major: col = s*B + b -> valid.T flattened
    valid_cells = np.ascontiguousarray(valid.T).reshape(1, CELLS)
    valid_bc = np.broadcast_to(valid_cells, (128, CELLS)).astype(np.float32).copy()
    woutT = np.ascontiguousarray(W_out.T)
    in_maps2 = []
    for c in range(NC):
        vs = slice(c * VSL, (c + 1) * VSL)
        in_maps2.append({
            "gi_seq": gi_seq,
            "zt1": zt1,
            "wl2ht": wl2ht,
            "whht_hi": whh_hi,
            "whht_lo": whh_lo,
            "woutt": np.ascontiguousarray(woutT[:, vs]),
            "bout_bc": np.broadcast_to(b_out[vs][None, :], (128, VSL)).astype(np.float32).copy(),
            "valid_bc": valid_bc,
            "bhh_n": b_hh[2 * H :][None, :].astype(np.float32),
        })
    r2 = run_bass_kernel_spmd(k2, in_maps2, core_ids=list(range(NC)), trace=TRACE)

    # ---- assemble outputs (cells are s-major: cell = s*B + b)
    p_cells = np.concatenate([r2.results[c]["p_out"] for c in range(NC)], axis=1)  # (2048, V)
    p_output = np.ascontiguousarray(p_cells.reshape(S, B, V).transpose(1, 0, 2))

    m_output = np.arange(S)[None, :] < length[:, None]

    outputs = r2.results[0]["outt_out"].T.astype(np.float64)  # (2048, H)
    global DEBUG, LAST_HW_NS
    DEBUG = {"outt": outputs, "p_cells": p_cells, "r1": r1, "r2": r2}
    if r1.exec_time_ns or r2.exec_time_ns:
        LAST_HW_NS = int((r1.exec_time_ns or 0) + (r2.exec_time_ns or 0))
    NCAND = 128
    sub = p_cells[:, 3:]
    cand = np.argpartition(-sub, NCAND, axis=1)[:, :NCAND] + 3
    Wc = W_out.astype(np.float64)[cand]
    logit64 = np.einsum("ch,cgh->cg", outputs, Wc) + b_out.astype(np.float64)[cand]
    ordk = np.lexsort((cand, -logit64), axis=1)[:, :MV]
    top_i = np.take_along_axis(cand, ordk, axis=1).astype(np.int32)
    top_v = np.take_along_axis(logit64, ordk, axis=1)
    s_cells = np.where(top_v > 0.0, top_i, np.int32(PAD_IDX)).astype(np.int32)
    s_output = np.ascontiguousarray(s_cells.reshape(S, B, MV).transpose(1, 0, 2))

    return (p_output.astype(np.float32), s_output, m_output)
